# revision 14
# baseline (speedup 1.0000x reference)
"""Causal GQA attention (nkv=1) with RoPE + logit softcap, sharded over 8 trn2 cores.

Sharding: core = 2*b + hh  (b = batch 0..3, hh = head-half 0..1).
Each core computes, for its batch b and its 4 query heads:
  q = rope(x @ Wq_h'.T)          (gain/(sqrt(hd)*softcap) folded into Wq on host)
  k = rope(x @ Wk.T), v = x @ Wv.T   (single kv head, shared across its 4 q heads)
  pT[k,q] = exp(softcap*tanh(qT.k) - softcap) * causal_mask   (max-free softmax:
            softcap bounds logits to +-30 so exp never overflows)
  outT_h = (v.T @ pT) / sum_k pT    accumulated in PSUM; denominator via ones-matmul
  partial_out[tok, :] = sum_h outT_h.T @ Wo[:, head cols].T
Host sums the two half-head partials per batch and stacks batches.

v3 (rewrite): ACT(scalar)-engine-minimal schedule. The kernel is jointly
PE/ACT-bound; tanh+exp over every computed score element is the floor. So:
  - diagonal groups use a PACKED score layout: k-block j only computes its
    visible q-columns [128j, 512), packed contiguously -> 1280 instead of
    2048 columns through matmul, tanh and exp.
  - tanh is ONE activation per group reading a 4-bank [128, 2048] fp32 PSUM
    tile (s_pool), exp is one activation -> p4 bf16.
  - softmax denominator: quad-compress p4 on DVE (3 adds) then a single
    ones-matmul per group (PE cost halved vs pair-compress).
  - V projected directly as [tok, hd] (x-tile stationary), no PE transpose.
  - Wo partials DMA'd to DRAM straight from PSUM (no SBUF staging copy).
  - global software pipeline: AV/denominator lag their scores by one task;
    Q/K/V projections and the previous chunk's Wo run as PE filler inside
    each chunk's ACT-bound stretch. DMAs prioritized so the first tanh
    starts ~12us in.
All matmuls bf16 (1 cyc/row); scores accumulate fp32 in PSUM; tanh keeps
fp32 until the bf16 exp output.
"""
import numpy as np
import ml_dtypes

import concourse.bacc as bacc
import concourse.mybir as mybir
import concourse.tile as tile
from concourse.bass_utils import run_bass_kernel_spmd

F32 = mybir.dt.float32
BF16 = mybir.dt.bfloat16
NPBF16 = ml_dtypes.bfloat16

B, T, D = 4, 2048, 1024
NH, NKV, HD = 8, 1, 128
SOFTCAP = 30.0
NHL = 4            # heads per core
CH = 512           # q-chunk size
NCH = T // CH      # 4 chunks
NKT = D // 128     # 8 k-tiles over D
NTT = T // 128     # 16 token tiles

# packed column offsets for diagonal groups: k-block j holds its visible
# q-columns [128j, 512) at [DOFF[j], DOFF[j+1]) — padded so every matmul
# output stays inside one 2KB PSUM bank (512 fp32). [896, 1024) is unused.
DOFF = [0, 512, 1024, 1280, 1408]
DW = 4 * CH        # score tile width (non-diag)


def _build_nc():
    nc = bacc.Bacc()

    xT = nc.dram_tensor("xT", [D, T], BF16, kind="ExternalInput")
    wqT = nc.dram_tensor("wqT", [D, NHL * HD], BF16, kind="ExternalInput")
    wkT = nc.dram_tensor("wkT", [D, HD], BF16, kind="ExternalInput")
    wvT = nc.dram_tensor("wvT", [D, HD], BF16, kind="ExternalInput")
    woT = nc.dram_tensor("woT", [NHL * HD, D], BF16, kind="ExternalInput")
    cc = nc.dram_tensor("cc", [HD, T], BF16, kind="ExternalInput")
    ssw = nc.dram_tensor("ssw", [HD, T], BF16, kind="ExternalInput")
    tri = nc.dram_tensor("tri", [128, 128], BF16, kind="ExternalInput")
    onesv = nc.dram_tensor("onesv", [128, 128], BF16, kind="ExternalInput")
    out = nc.dram_tensor("out", [T, D], F32, kind="ExternalOutput")

    xT_t = xT.rearrange("(kt p) t -> p kt t", p=128)      # [128, 8, 2048]
    wqT_t = wqT.rearrange("(kt p) c -> p kt c", p=128)    # [128, 8, 512]
    wkT_t = wkT.rearrange("(kt p) c -> p kt c", p=128)    # [128, 8, 128]
    wvT_t = wvT.rearrange("(kt p) c -> p kt c", p=128)    # [128, 8, 128]
    woT_t = woT.rearrange("(h p) c -> p h c", p=128)      # [128, 4, 1024]

    with tile.TileContext(nc) as tc:
        with (
            tc.tile_pool(name="persist", bufs=1) as persist,
            tc.tile_pool(name="wpool", bufs=1) as wpool,
            tc.tile_pool(name="qt_pool", bufs=2) as qt_pool,
            tc.tile_pool(name="rope_pool", bufs=2) as rope_pool,
            tc.tile_pool(name="p_pool", bufs=3) as p_pool,
            tc.tile_pool(name="pp_pool", bufs=2) as pp_pool,
            tc.tile_pool(name="t4_pool", bufs=1) as t4_pool,
            tc.tile_pool(name="avn_pool", bufs=12) as avn_pool,
            tc.tile_pool(name="osb_pool", bufs=3) as osb_pool,
            tc.tile_pool(name="norm_pool", bufs=2) as norm_pool,
            tc.tile_pool(name="s_pool", bufs=1, space="PSUM") as s_pool,
            tc.tile_pool(name="acc_pool", bufs=1, space="PSUM") as acc_pool,
            tc.tile_pool(name="d_pool", bufs=1, space="PSUM") as d_pool,
            tc.tile_pool(name="pj_pool", bufs=2, space="PSUM") as pj_pool,
        ):
            # --- persistent tiles ---
            wq_sb = [wpool.tile([128, NHL * HD], BF16, name=f"wq{kt}")
                     for kt in range(NKT)]
            wk_sb = wpool.tile([128, NKT, HD], BF16)
            wv_sb = wpool.tile([128, NKT, HD], BF16)
            wo_sb = wpool.tile([128, NHL, D], BF16)
            cc_sb = wpool.tile([HD, T], BF16)
            ssw_sb = wpool.tile([HD, T], BF16)
            tri_sb = wpool.tile([128, 128], BF16)
            ones_sb = wpool.tile([128, 128], BF16)
            xT_sb = wpool.tile([128, NKT, T], BF16)
            kT_sb = persist.tile([HD, T], BF16)
            v_sb = persist.tile([128, NTT, HD], BF16)
            negcap = persist.tile([128, 1], F32)
            nc.gpsimd.memset(negcap[:], -SOFTCAP)

            # --- DMA priorities ---
            # sync HWDGE: wq/x-chunk0 pairs (q/k proj of chunk 0 unblocks at
            # pair kt), then the remaining x chunks. out DMAs ride this queue
            # later (program order after these).
            for kt in range(NKT):
                nc.sync.dma_start(wq_sb[kt][:], wqT_t[:, kt, :])
                nc.sync.dma_start(xT_sb[:, kt, 0:CH], xT_t[:, kt, 0:CH])
            # scalar HWDGE: small operands the first rope/scores/mask need;
            # all issued before attention's ACT work starts.
            nc.scalar.dma_start(wk_sb[:], wkT_t)
            nc.scalar.dma_start(cc_sb[:, 0:CH], cc[:, 0:CH])
            nc.scalar.dma_start(ssw_sb[:, 0:CH], ssw[:, 0:CH])
            nc.scalar.dma_start(wv_sb[:], wvT_t)
            nc.scalar.dma_start(tri_sb[:], tri[:])
            nc.scalar.dma_start(ones_sb[:], onesv[:])
            nc.scalar.dma_start(cc_sb[:, CH:T], cc[:, CH:T])
            nc.scalar.dma_start(ssw_sb[:, CH:T], ssw[:, CH:T])
            nc.scalar.dma_start(wo_sb[:], woT_t)
            for c in range(1, NCH):
                for kt in range(NKT):
                    nc.sync.dma_start(xT_sb[:, kt, c * CH:(c + 1) * CH],
                                      xT_t[:, kt, c * CH:(c + 1) * CH])

            def rope_to(dst_ap, src_ps, c):
                """dst = rope(src) for a [128, CH] chunk at token offset c*CH.

                All-bf16 after the PSUM read; partition half-swap must go
                through tensor_copy (TT ops need aligned partitions). PSUM
                reads stay on DVE; the SBUF-only mul/mul/add run on the
                otherwise-idle gpsimd to keep DVE below the ACT floor."""
                csl = slice(c * CH, (c + 1) * CH)
                qb = rope_pool.tile([128, CH], BF16, tag="qb", name="qb")
                nc.vector.tensor_copy(qb[:], src_ps[:])
                swp = rope_pool.tile([128, CH], BF16, tag="swp", name="swp")
                nc.vector.tensor_copy(swp[0:64, :], qb[64:128, :])
                nc.vector.tensor_copy(swp[64:128, :], qb[0:64, :])
                m1 = rope_pool.tile([128, CH], BF16, tag="m1", name="m1")
                nc.gpsimd.tensor_mul(m1[:], qb[:], cc_sb[:, csl])
                m2 = rope_pool.tile([128, CH], BF16, tag="m2", name="m2")
                nc.gpsimd.tensor_mul(m2[:], swp[:], ssw_sb[:, csl])
                nc.gpsimd.tensor_add(dst_ap, m1[:], m2[:])

            # ---- filler units (pure-PE work scheduled into ACT-bound gaps) ----
            qt_tiles = {}     # c -> qt tile [HD, NHL, CH]

            def qp_unit(c, h):
                csl = slice(c * CH, (c + 1) * CH)
                if h == 0:
                    qt_tiles[c] = qt_pool.tile([HD, NHL, CH], BF16, tag="qt",
                                               name="qt")
                q_ps = pj_pool.tile([128, CH], F32, tag="pj", name="q_ps")
                for kt in range(NKT):
                    nc.tensor.matmul(
                        q_ps[0:HD, :], wq_sb[kt][:, h * HD:(h + 1) * HD],
                        xT_sb[:, kt, csl], start=(kt == 0), stop=(kt == NKT - 1))
                rope_to(qt_tiles[c][:, h, :], q_ps[0:HD, :], c)

            def kp_unit(c):
                csl = slice(c * CH, (c + 1) * CH)
                k_ps = pj_pool.tile([128, CH], F32, tag="pj", name="k_ps")
                for kt in range(NKT):
                    nc.tensor.matmul(k_ps[0:HD, :], wk_sb[:, kt, :],
                                     xT_sb[:, kt, csl],
                                     start=(kt == 0), stop=(kt == NKT - 1))
                rope_to(kT_sb[:, csl], k_ps[0:HD, :], c)

            def vp_unit(c, tt):
                # V directly as [tok, hd]: x-tile stationary, wv moving.
                tsl = slice((c * 4 + tt) * 128, (c * 4 + tt + 1) * 128)
                v_ps = pj_pool.tile([128, CH], F32, tag="pj", name="v_ps")
                for kt in range(NKT):
                    nc.tensor.matmul(v_ps[:, 0:HD], xT_sb[:, kt, tsl],
                                     wv_sb[:, kt, :],
                                     start=(kt == 0), stop=(kt == NKT - 1))
                nc.vector.tensor_copy(v_sb[:, c * 4 + tt, :], v_ps[:, 0:HD])

            avn_tiles = {}    # (c, h) -> avn tile

            def wo_unit(c, u):
                tt, dc = u // 2, u % 2
                o_ps = pj_pool.tile([128, CH], F32, tag="pj", name="o_ps")
                for h in range(NHL):
                    nc.tensor.matmul(
                        o_ps[:], avn_tiles[(c, h)][:, tt * 128:(tt + 1) * 128],
                        wo_sb[:, h, dc * CH:(dc + 1) * CH],
                        start=(h == 0), stop=(h == NHL - 1))
                # DMA can't source PSUM (and neither can gpsimd): stage on DVE
                o_sb = osb_pool.tile([128, CH], F32, tag="osb", name="o_sb")
                nc.vector.tensor_copy(o_sb[:], o_ps[:])
                nc.sync.dma_start(
                    out[c * CH + tt * 128: c * CH + (tt + 1) * 128,
                        dc * CH:(dc + 1) * CH], o_sb[:])

            # ---- filler scheduling ----
            emitted = set()

            def emit_unit(u):
                if u in emitted:
                    return
                emitted.add(u)
                kind = u[0]
                if kind == "qp":
                    qp_unit(u[1], u[2])
                elif kind == "kp":
                    kp_unit(u[1])
                elif kind == "vp":
                    vp_unit(u[1], u[2])
                elif kind == "wo":
                    wo_unit(u[1], u[2])

            # per-chunk filler lists: chunk c runs proj(c+1) and Wo(c-1)
            fillers = {}
            fillers[-1] = ([("vp", 0, tt) for tt in range(4)]
                           + [("qp", 0, h) for h in range(1, NHL)])
            for c in range(NCH):
                f = []
                if c + 1 < NCH:
                    f += ([("qp", c + 1, h) for h in range(NHL)]
                          + [("kp", c + 1)]
                          + [("vp", c + 1, tt) for tt in range(4)])
                if c >= 1:
                    f += [("wo", c - 1, u) for u in range(8)]
                fillers[c] = f

            # ---- attention task machinery ----
            pend = [None]   # lagged AV work: (c, h, g, p4_tile)
            head_acc = {}   # (c, h) -> (av_ps, d_ps), allocated at g == 0

            def emit_av(c, h, g, p4):
                """AV + quad-compress + ones-matmul for task (c,h,g); the
                consuming accumulators live across the head's groups."""
                diag = g == c
                for tt in range(4):
                    emit_unit(("vp", g, tt))
                if g == 0:
                    av_ps = acc_pool.tile([HD, CH], F32, tag="av", name="av_ps")
                    d_ps = d_pool.tile([128, CH], F32, tag="d", name="d_ps")
                    head_acc[(c, h)] = (av_ps, d_ps)
                av_ps, d_ps = head_acc[(c, h)]
                for j in range(4):
                    kb = 4 * g + j
                    if diag:
                        lo, po = 128 * j, DOFF[j]
                        w = CH - lo
                        nc.tensor.matmul(av_ps[:, lo:CH], v_sb[:, kb, :],
                                         p4[:, po:po + w],
                                         start=(kb == 0),
                                         stop=(g == c and j == 3))
                    else:
                        nc.tensor.matmul(av_ps[:], v_sb[:, kb, :],
                                         p4[:, j * CH:(j + 1) * CH],
                                         start=(kb == 0), stop=False)
                # quad-compress for the denominator: 3 DVE adds -> 1 ones-MM
                ppq = pp_pool.tile([128, CH], BF16, tag="ppq", name="ppq")
                if diag:
                    nc.vector.tensor_copy(ppq[:], p4[:, 0:CH])
                    for j in range(1, 4):
                        lo = 128 * j
                        nc.vector.tensor_add(
                            ppq[:, lo:CH], ppq[:, lo:CH],
                            p4[:, DOFF[j]:DOFF[j] + (CH - lo)])
                else:
                    ppa = pp_pool.tile([128, CH], BF16, tag="ppa", name="ppa")
                    nc.vector.tensor_add(ppa[:], p4[:, 0:CH], p4[:, CH:2 * CH])
                    ppb = pp_pool.tile([128, CH], BF16, tag="ppb", name="ppb")
                    nc.vector.tensor_add(ppb[:], p4[:, 2 * CH:3 * CH],
                                         p4[:, 3 * CH:4 * CH])
                    nc.vector.tensor_add(ppq[:], ppa[:], ppb[:])
                nc.tensor.matmul(d_ps[:], ones_sb[:], ppq[:],
                                 start=(g == 0), stop=(g == c))
                if g == c:
                    # head (c,h) complete: normalize
                    dinv = norm_pool.tile([128, CH], F32, tag="dinv",
                                          name="dinv")
                    nc.vector.reciprocal_approx_fast(dinv[:], d_ps[:])
                    avn = avn_pool.tile([HD, CH], BF16, tag="avn", name="avn")
                    nc.vector.tensor_mul(avn[:], av_ps[:], dinv[:])
                    avn_tiles[(c, h)] = avn

            def emit_scores(c, h, g):
                """scores -> tanh -> exp(-> tri mask) for task (c,h,g)."""
                diag = g == c
                emit_unit(("kp", g))
                qt = qt_tiles[c]
                s_t = s_pool.tile([128, DW], F32, tag="s", name="s_t")
                t4 = t4_pool.tile([128, DW], F32, tag="t4", name="t4")
                p4 = p_pool.tile([128, DW], BF16, tag="p4", name="p4")
                if diag:
                    for j in range(4):
                        kb = 4 * g + j
                        lo, po = 128 * j, DOFF[j]
                        w = CH - lo
                        nc.tensor.matmul(
                            s_t[:, po:po + w],
                            kT_sb[:, kb * 128:(kb + 1) * 128],
                            qt[:, h, lo:CH], start=True, stop=True)
                    nw = DOFF[4]
                else:
                    for j in range(4):
                        kb = 4 * g + j
                        nc.tensor.matmul(
                            s_t[:, j * CH:(j + 1) * CH],
                            kT_sb[:, kb * 128:(kb + 1) * 128],
                            qt[:, h, :], start=True, stop=True)
                    nw = DW
                nc.scalar.activation(t4[:, 0:nw], s_t[:, 0:nw],
                                     mybir.ActivationFunctionType.Tanh)
                nc.scalar.activation(p4[:, 0:nw], t4[:, 0:nw],
                                     mybir.ActivationFunctionType.Exp,
                                     scale=SOFTCAP, bias=negcap[:])
                if diag:
                    # mask the four partially-visible 128-col triangles
                    for j in range(4):
                        po = DOFF[j]
                        nc.gpsimd.tensor_mul(p4[:, po:po + 128],
                                             p4[:, po:po + 128], tri_sb[:])
                return p4

            # ---- main schedule ----
            # prologue: just enough for the first task
            emit_unit(("qp", 0, 0))
            emit_unit(("kp", 0))

            for c in range(NCH):
                if c == 0:
                    # chunk-0's own proj rides the filler slots so the first
                    # scores/tanh start as early as possible
                    flist = fillers[-1] + fillers[0]
                else:
                    # leftover fillers from the previous chunk must land
                    # (kp/vp/qp of this chunk among them)
                    for u in fillers[c - 1]:
                        emit_unit(u)
                    flist = fillers[c]
                ntasks = NHL * (c + 1)
                nf = len(flist)
                ti = 0
                for h in range(NHL):
                    emit_unit(("qp", c, h))
                    for g in range(c + 1):
                        p4 = emit_scores(c, h, g)
                        if pend[0] is not None:
                            emit_av(*pend[0])
                        pend[0] = (c, h, g, p4)
                        # spread this chunk's fillers evenly across tasks
                        lo = (ti * nf) // ntasks
                        hi = ((ti + 1) * nf) // ntasks
                        for u in flist[lo:hi]:
                            emit_unit(u)
                        ti += 1
            emit_av(*pend[0])
            for u in fillers[NCH - 1]:
                emit_unit(u)
            for u in range(8):
                emit_unit(("wo", NCH - 1, u))

    nc.compile()
    return nc


_CACHED_NC = None


def _get_nc():
    global _CACHED_NC
    if _CACHED_NC is None:
        _CACHED_NC = _build_nc()
    return _CACHED_NC


def _host_inputs(x, Wq, Wk, Wv, Wo, qk_gain, cos, sin):
    """Build the 8 per-core input maps (bf16 matmul operands)."""
    x = np.asarray(x, np.float32)
    Wq = np.asarray(Wq, np.float32)
    Wk = np.asarray(Wk, np.float32)
    Wv = np.asarray(Wv, np.float32)
    Wo = np.asarray(Wo, np.float32)
    qk_gain = np.asarray(qk_gain, np.float32)
    cos = np.asarray(cos, np.float32)
    sin = np.asarray(sin, np.float32)

    scale = 1.0 / (np.sqrt(HD) * SOFTCAP)
    # Fold per-head gain and softcap scale into Wq rows.
    Wq_s = Wq * (qk_gain[:, None].repeat(HD, 1).reshape(NH * HD, 1) * scale)

    wkT = np.ascontiguousarray(Wk.T.astype(NPBF16))
    wvT = np.ascontiguousarray(Wv.T.astype(NPBF16))
    cosT = cos.T  # [64, T]
    sinT = sin.T
    cc = np.ascontiguousarray(np.concatenate([cosT, cosT], 0).astype(NPBF16))
    # m2 = swap(q) * ssw with swap done via copies: ssw = [-sin; sin]
    ssw = np.ascontiguousarray(np.concatenate([-sinT, sinT], 0).astype(NPBF16))

    # triangular mask for the diagonal 128-blocks: tri[kk, qq] = qq >= kk
    kk = np.arange(128)
    tri = (kk[None, :] >= kk[:, None]).astype(NPBF16)
    onesv = np.ones((128, 128), NPBF16)

    xTs = [np.ascontiguousarray(x[b].T.astype(NPBF16)) for b in range(B)]
    in_maps = []
    for core in range(8):
        b, hh = divmod(core, 2)
        h0 = hh * NHL
        wqT = np.ascontiguousarray(
            Wq_s[h0 * HD:(h0 + NHL) * HD, :].T.astype(NPBF16))
        woT = np.ascontiguousarray(
            Wo[:, h0 * HD:(h0 + NHL) * HD].T.astype(NPBF16))
        in_maps.append({
            "xT": xTs[b], "wqT": wqT, "wkT": wkT, "wvT": wvT, "woT": woT,
            "cc": cc, "ssw": ssw, "tri": tri, "onesv": onesv,
        })
    return in_maps


def kernel(x, Wq, Wk, Wv, Wo, qk_gain, cos, sin, _trace=False):
    in_maps = _host_inputs(x, Wq, Wk, Wv, Wo, qk_gain, cos, sin)
    nc = _get_nc()
    res = run_bass_kernel_spmd(nc, in_maps, core_ids=list(range(8)),
                               trace=_trace)
    out = np.empty((B, T, D), np.float32)
    for b in range(B):
        out[b] = res.results[2 * b]["out"] + res.results[2 * b + 1]["out"]
    if _trace:
        kernel.last_exec_time_ns = res.exec_time_ns
        kernel.last_results = res
    return out
